# revision 29
# baseline (speedup 1.0000x reference)
"""Trainium2 Bass kernel for nn_ATM (token merging w/ DPC-KNN clustering).

Fully data-parallel over batch B=8: one sample per NeuronCore.
Heavy compute on device: head-sum, two Gram matmuls (x and attn cdist),
distance assembly, 5-NN density, rank-based top-256 center selection,
symmetric-argmin cluster assignment, one-hot matmul scatter-merge.

Self-contained: hardcodes all shapes from the problem spec.
"""

import numpy as np
from contextlib import ExitStack

import concourse.bass as bass
import concourse.bacc as bacc
import concourse.mybir as mybir
import concourse.tile as tile
from concourse import masks
import concourse.bass_isa as bass_isa
from concourse.bass_types import AP

F32 = mybir.dt.float32
BF16 = mybir.dt.bfloat16
F32R = mybir.dt.float32r
ALU = mybir.AluOpType
ACTF = mybir.ActivationFunctionType

# problem constants
B = 8
N = 1024          # tokens
R = 8             # token chunks of 128
CIN = 128
COUT = 256
CL = 256          # cluster num
HEADS = 4
GRID = 32         # init grid (32x32)
OG = 16           # conv output grid (16x16)
KNN = 5
BIG = 1.0e6
SCALE = 1.0 / 16.0            # 1/sqrt(256)
WX = 0.8 * SCALE              # weight on cdist(x)
WA = 0.2 * SCALE              # weight on cdist(attn)
M2T = 1.0 / (1.0 + 1e-6)      # map2token count division

# matmul input dtype: float32 (safe) or float32r (4x faster, N>=256)
MM_DT = mybir.dt.float32
DEBUG = False


def _mm(ap):
    if MM_DT == F32:
        return ap
    return ap.bitcast(MM_DT)


def _sub_ap(t_ap, extra_off, dims):
    return AP(t_ap.tensor, t_ap.offset + extra_off, dims)


def build_nc():
    nc = bacc.Bacc("TRN2", target_bir_lowering=False, debug=False)

    # ---- dram tensors (per-core inputs) ----
    d_x = nc.dram_tensor("x", [N, CIN], F32, kind="ExternalInput").ap()
    d_attn = nc.dram_tensor("attn", [HEADS, N, N], F32, kind="ExternalInput").ap()
    d_asn = nc.dram_tensor("asn", [128, R], F32, kind="ExternalInput").ap()
    d_noise = nc.dram_tensor("noise", [128, R], F32, kind="ExternalInput").ap()
    d_convw = nc.dram_tensor("convw", [128, 9 * COUT], F32, kind="ExternalInput").ap()
    d_convb = nc.dram_tensor("convb", [128, 2], F32, kind="ExternalInput").ap()
    d_skipw = nc.dram_tensor("skipw", [128, COUT], F32, kind="ExternalInput").ap()
    d_lng = nc.dram_tensor("lng", [128, COUT], F32, kind="ExternalInput").ap()
    d_lnb = nc.dram_tensor("lnb", [128, COUT], F32, kind="ExternalInput").ap()
    d_scw = nc.dram_tensor("scw", [128, COUT], F32, kind="ExternalInput").ap()
    d_scb = nc.dram_tensor("scb", [128, 1], F32, kind="ExternalInput").ap()
    d_iota = nc.dram_tensor("iota", [128, COUT], F32, kind="ExternalInput").ap()
    d_out = nc.dram_tensor("out", [CL, COUT], F32, kind="ExternalOutput").ap()
    d_convT = nc.dram_tensor("convT_scratch", [OG * OG, COUT], F32).ap()

    dbg = {}
    if DEBUG:
        for nm, shp in [
            ("dbg_x1", [R, 128, COUT]), ("dbg_xn", [R, 128, COUT]),
            ("dbg_cols", [128, R * 8]), ("dbg_dist0", [128, N]),
        ]:
            dbg[nm] = nc.dram_tensor(nm, shp, F32, kind="ExternalOutput").ap()

    with tile.TileContext(nc) as tc, ExitStack() as ctx:
        # ---------- persistent pools ----------
        pp = ctx.enter_context(tc.tile_pool(name="persist", bufs=1))
        xT = pp.tile([128, N], F32)               # x transposed [ci, tok]
        xnT = pp.tile([128, 2, N], F32)           # xn transposed [ch, tok]
        attnT_hi = pp.tile([128, R, N], BF16)     # attn^T bf16 high part
        attnT_lo = pp.tile([128, R, N], BF16)     # attn^T bf16 low part
        dist = pp.tile([128, R, N], F32)          # dist[i-chunk][i_p, j]
        xn = pp.tile([128, R, COUT], F32)         # normed tokens
        # column vectors [128, R] etc.
        sqx_c = pp.tile([128, R], F32)
        sqa_c = pp.tile([128, R], F32)
        den_c = pp.tile([128, R], F32)
        dmin_c = pp.tile([128, R], F32)
        sco_c = pp.tile([128, R], F32)
        rank_c = pp.tile([128, R], F32)
        cid_c = pp.tile([128, R], F32)
        tw_c = pp.tile([128, R], F32)
        dmx_c = pp.tile([128, R], F32)
        dmax_a = pp.tile([128, 1], F32)
        # weights / constants
        w_conv = pp.tile([128, 9 * COUT], F32)
        w_skip = pp.tile([128, COUT], F32)
        w_lng = pp.tile([128, COUT], F32)
        w_lnb = pp.tile([128, COUT], F32)
        w_scw = pp.tile([128, COUT], F32)
        w_scb = pp.tile([128, 1], F32)
        w_cb = pp.tile([128, 2], F32)
        t_asn = pp.tile([128, R], F32)
        t_noise = pp.tile([128, R], F32)
        ident = pp.tile([128, 128], F32)
        eps_ln = pp.tile([128, 1], F32)
        ones1 = pp.tile([32, 128], F32)
        iota256 = pp.tile([128, COUT], F32)

        # [128, N] broadcast rows — two reusable slots
        rows = ctx.enter_context(tc.tile_pool(name="rows", bufs=1))
        # big scratch [128, <=2048], shared tag
        sb = ctx.enter_context(tc.tile_pool(name="sbig", bufs=6))
        # medium/small scratch
        sp = ctx.enter_context(tc.tile_pool(name="scratch", bufs=2))
        jp = ctx.enter_context(tc.tile_pool(name="junk", bufs=2))

        _bigctr = [0]

        def big(shape):
            _bigctr[0] += 1
            return sb.tile(shape, F32, tag="big", name=f"big{_bigctr[0]}")

        # ---------- load constants ----------
        nc.sync.dma_start(w_conv[:], d_convw)
        nc.sync.dma_start(w_skip[:], d_skipw)
        nc.sync.dma_start(w_lng[:], d_lng)
        nc.sync.dma_start(w_lnb[:], d_lnb)
        nc.sync.dma_start(w_scw[:], d_scw)
        nc.sync.dma_start(w_scb[:], d_scb)
        nc.sync.dma_start(w_cb[:], d_convb)
        nc.sync.dma_start(t_asn[:], d_asn)
        nc.sync.dma_start(t_noise[:], d_noise)
        masks.make_identity(nc, ident[:])
        nc.gpsimd.memset(eps_ln[:], 1e-5)
        nc.sync.dma_start(iota256[:], d_iota)
        nc.gpsimd.memset(ones1[:], 1.0)

        # ---------- helpers ----------
        _c2rctr = [0]

        def col2row(col_ap, psum_pool):
            """[128, R] column layout -> [1, N] row (token order)."""
            _c2rctr[0] += 1
            i = _c2rctr[0]
            pt = psum_pool.tile([32, 128], F32, tag="c2r", name=f"c2rp{i}")
            nc.tensor.matmul(pt[0:R, :], col_ap, ident[:], is_transpose=True)
            srow = sp.tile([32, 128], F32, tag="c2r_s", name=f"c2rs{i}")
            nc.vector.tensor_copy(srow[0:R, :], pt[0:R, :])
            row1 = sp.tile([32, N], F32, tag="c2row", name=f"c2rr{i}")
            nc.sync.dma_start(
                _sub_ap(row1[:], 0, [[N, 1], [1, N]]),
                _sub_ap(srow[:], 0, [[128, R], [1, 128]]),
            )
            return row1

        def col2rowb(col_ap, psum_pool, row_tag):
            """[128, R] column layout -> [128, N] partition-broadcast rows."""
            row1 = col2row(col_ap, psum_pool)
            row_b = rows.tile([128, N], F32, tag=row_tag,
                              name=f"rowb{_c2rctr[0]}")
            for h in range(2):
                pb = psum_pool.tile([128, 512], F32, tag="bc",
                                    name=f"c2rb{_c2rctr[0]}_{h}")
                nc.tensor.matmul(pb[:], ones1[0:1, :],
                                 row1[0:1, h * 512:(h + 1) * 512],
                                 start=True, stop=True)
                nc.scalar.copy(row_b[:, h * 512:(h + 1) * 512], pb[:])
            return row_b

        # =========== phase 1: x side ===========
        with tc.tile_pool(name="pt", bufs=2, space="PSUM") as ptp, \
             tc.tile_pool(name="pcv", bufs=1, space="PSUM") as pcv, \
             tc.tile_pool(name="psk", bufs=2, space="PSUM") as psk:

            # x -> xT via PE transpose (packed 4 per psum tile)
            for g in range(2):
                pt = ptp.tile([128, 512], F32, tag="tp")
                for rr in range(4):
                    r = g * 4 + rr
                    xc = sp.tile([128, CIN], F32, tag="xc")
                    eng = nc.sync if r % 2 == 0 else nc.scalar
                    eng.dma_start(xc[:], d_x[r * 128:(r + 1) * 128, :])
                    nc.tensor.matmul(pt[:, rr * 128:(rr + 1) * 128], xc[:],
                                     ident[:], is_transpose=True)
                nc.vector.tensor_copy(xT[:, g * 512:(g + 1) * 512], pt[:])

            # conv (3x3 stride2 pad1) as 9-tap im2col matmuls on a
            # zero-padded 34x34 map so every tap covers the full 16x16 out
            PAD = 34
            xpad = sp.tile([128, PAD * PAD], F32, tag="xpad")
            nc.vector.memset(xpad[:], 0.0)
            nc.vector.tensor_copy(
                _sub_ap(xpad[:], PAD + 1, [[PAD * PAD, 128], [PAD, 32], [1, 32]]),
                xT[:].rearrange("c (h w) -> c h w", h=32, w=32))
            pc0 = pcv.tile([128, OG * OG], F32, tag="pc0")
            pc1 = pcv.tile([128, OG * OG], F32, tag="pc1")
            pcs = [pc0, pc1]
            for t in range(9):
                dy, dx = t // 3, t % 3
                rhs = _sub_ap(xpad[:], dy * PAD + dx,
                              [[PAD * PAD, 128], [2 * PAD, OG], [2, OG]])
                for cm in range(2):
                    lhsT = w_conv[:, t * COUT + cm * 128: t * COUT + (cm + 1) * 128]
                    nc.tensor.matmul(pcs[cm][:], _mm(lhsT), _mm(rhs),
                                     start=(t == 0), stop=(t == 8))

            # conv psum -> sbuf (+bias), transpose to [pos, co], bounce to DRAM
            for cm in range(2):
                csb = sp.tile([128, OG * OG], F32, tag="csb")
                nc.scalar.activation(csb[:], pcs[cm][:], ACTF.Identity,
                                     bias=w_cb[:, cm:cm + 1], scale=1.0)
                for pm in range(2):
                    pt = ptp.tile([128, 128], F32, tag="tp")
                    nc.tensor.matmul(pt[:], csb[:, pm * 128:(pm + 1) * 128],
                                     ident[:], is_transpose=True)
                    ctile = sp.tile([128, 128], F32, tag="ctile")
                    nc.vector.tensor_copy(ctile[:], pt[:])
                    nc.sync.dma_start(
                        d_convT[pm * 128:(pm + 1) * 128,
                                cm * 128:(cm + 1) * 128], ctile[:])

            # per token chunk: skip matmul + upsampled conv + LN + score
            for r in range(R):
                pskt = psk.tile([128, COUT], F32, tag="psk")
                nc.tensor.matmul(pskt[:], _mm(xT[:, r * 128:(r + 1) * 128]),
                                 _mm(w_skip[:]), start=True, stop=True)
                up = sp.tile([128, COUT], F32, tag="up")
                # gather rows (i//2)*16 + (j//2) for tokens of chunk r
                for i2 in range(2):
                    for irep in range(2):
                        il = i2 * 2 + irep
                        src = _sub_ap(d_convT, (2 * r + i2) * OG * COUT,
                                      [[COUT, OG], [0, 2], [1, COUT]])
                        eng = nc.sync if il % 2 == 0 else nc.scalar
                        eng.dma_start(up[il * 32:(il + 1) * 32, :], src)
                x1 = sp.tile([128, COUT], F32, tag="x1")
                nc.vector.scalar_tensor_tensor(
                    x1[:], up[:], M2T, pskt[:], op0=ALU.mult, op1=ALU.add)
                if DEBUG:
                    nc.sync.dma_start(dbg["dbg_x1"][r], x1[:])
                # layernorm
                mu = sp.tile([128, 1], F32, tag="mu")
                nc.vector.tensor_reduce(mu[:], x1[:], axis=mybir.AxisListType.X,
                                        op=ALU.add)
                nc.vector.tensor_scalar_mul(mu[:], mu[:], 1.0 / COUT)
                tcen = sp.tile([128, COUT], F32, tag="tcen")
                nc.vector.tensor_scalar_sub(tcen[:], x1[:], mu[:])
                j1k = jp.tile([128, N], F32, tag="j1k")
                var = sp.tile([128, 1], F32, tag="var")
                nc.scalar.activation(j1k[:, 0:COUT], tcen[:], ACTF.Square,
                                     accum_out=var[:])
                sd = sp.tile([128, 1], F32, tag="sd")
                nc.scalar.activation(sd[:], var[:], ACTF.Sqrt,
                                     bias=eps_ln[:], scale=1.0 / COUT)
                rstd = sp.tile([128, 1], F32, tag="rstd")
                nc.vector.reciprocal(rstd[:], sd[:])
                xnr = xn[:, r, :]
                nc.scalar.activation(xnr, tcen[:], ACTF.Copy, scale=rstd[:])
                nc.vector.tensor_tensor(xnr, xnr, w_lng[:], op=ALU.mult)
                nc.vector.tensor_tensor(xnr, xnr, w_lnb[:], op=ALU.add)
                if DEBUG:
                    nc.sync.dma_start(dbg["dbg_xn"][r], xnr)
                # token weight exp(score)
                scr = sp.tile([128, 1], F32, tag="scr")
                nc.vector.scalar_tensor_tensor(
                    j1k[:, 0:COUT], xnr, 1.0, w_scw[:], op0=ALU.mult,
                    op1=ALU.mult, accum_out=scr[:])
                nc.scalar.activation(tw_c[:, r:r + 1], scr[:], ACTF.Exp,
                                     bias=w_scb[:, 0:1], scale=1.0)
                # sum of squares
                nc.scalar.activation(j1k[:, 0:COUT], xnr, ACTF.Square,
                                     accum_out=sqx_c[:, r:r + 1])
                # xn transpose chunks
                ptx = ptp.tile([128, 2, 128], F32, tag="tp")
                for km in range(2):
                    nc.tensor.matmul(
                        ptx[:, km, :],
                        xnr.rearrange("p (k c) -> p k c", k=2, c=128)[:, km, :],
                        ident[:], is_transpose=True)
                nc.vector.tensor_copy(xnT[:, :, r * 128:(r + 1) * 128], ptx[:])

        # =========== G_x during attn loads; then attn; then G_a ===========
        with tc.tile_pool(name="pc2r", bufs=1, space="PSUM") as p2r, \
             tc.tile_pool(name="pg", bufs=2, space="PSUM") as pg:
            sqx_rb = col2rowb(sqx_c[:], p2r, "rowA")

            def segs_of(m):
                cs0 = 128 * m
                segs = [(cs0, min(cs0 + 512, N)), (min(cs0 + 512, N), N)]
                return [(a, b) for (a, b) in segs if b > a]

            # ---- x-part of dist (upper triangle): dist = WX*sqrt(d2x) ----
            for m in range(R):
                for (cs, ce) in segs_of(m):
                    w = ce - cs
                    px = pg.tile([128, 512], F32, tag="px", bufs=1)
                    for k in range(2):
                        nc.tensor.matmul(
                            px[:, 0:w], _mm(xnT[:, k, m * 128:(m + 1) * 128]),
                            _mm(xnT[:, k, cs:ce]),
                            start=(k == 0), stop=(k == 1))
                    tx = sp.tile([128, 512], F32, tag="tx")
                    nc.scalar.activation(tx[:, 0:w], px[:, 0:w], ACTF.Identity,
                                         bias=sqx_c[:, m:m + 1], scale=-2.0)
                    nc.gpsimd.tensor_tensor(tx[:, 0:w], tx[:, 0:w],
                                            sqx_rb[:, cs:ce], op=ALU.add)
                    nc.gpsimd.tensor_scalar_max(tx[:, 0:w], tx[:, 0:w], 0.0)
                    nc.scalar.activation(dist[:, m, cs:ce], tx[:, 0:w],
                                         ACTF.Sqrt, scale=WX * WX)

            # ---- attn: load, head-sum, sq, transpose ----
            ptp2_cm = tc.tile_pool(name="pt2", bufs=2, space="PSUM")
            ptp2 = ptp2_cm.__enter__()
            for r in range(R):
                hts = []
                for h in range(HEADS):
                    ht = big([128, N])
                    eng = nc.sync if h % 2 == 0 else nc.scalar
                    eng.dma_start(
                        ht[:], d_attn[h, r * 128:(r + 1) * 128, :])
                    hts.append(ht)
                acc = big([128, N])
                nc.gpsimd.tensor_tensor(acc[:], hts[0][:], hts[1][:], op=ALU.add)
                nc.gpsimd.tensor_tensor(hts[2][:], hts[2][:], hts[3][:], op=ALU.add)
                nc.gpsimd.tensor_tensor(acc[:], acc[:], hts[2][:], op=ALU.add)
                j1k = jp.tile([128, N], F32, tag="j1k")
                nc.scalar.activation(j1k[:], acc[:], ACTF.Square,
                                     accum_out=sqa_c[:, r:r + 1])
                for g in range(2):
                    pt = ptp2.tile([128, 4, 128], F32, tag="tp2")
                    for kk in range(4):
                        k = g * 4 + kk
                        nc.tensor.matmul(pt[:, kk, :],
                                         acc[:, k * 128:(k + 1) * 128],
                                         ident[:], is_transpose=True)
                    hi_sl = attnT_hi[:, g * 4:(g + 1) * 4, r * 128:(r + 1) * 128]
                    nc.scalar.copy(hi_sl, pt[:])
                    nc.vector.tensor_tensor(
                        attnT_lo[:, g * 4:(g + 1) * 4, r * 128:(r + 1) * 128],
                        pt[:], hi_sl, op=ALU.subtract)

            ptp2_cm.__exit__(None, None, None)
            # ---- attn-part of dist: dist = -(tx + WA*sqrt(d2a)) ----
            sqa_rb = col2rowb(sqa_c[:], p2r, "rowB")
            for m in range(R):
                for (cs, ce) in segs_of(m):
                    w = ce - cs
                    pa = pg.tile([128, 512], F32, tag="pa", bufs=3)
                    for k in range(R):
                        him = attnT_hi[:, k, m * 128:(m + 1) * 128]
                        lom = attnT_lo[:, k, m * 128:(m + 1) * 128]
                        his = attnT_hi[:, k, cs:ce]
                        los = attnT_lo[:, k, cs:ce]
                        nc.tensor.matmul(pa[:, 0:w], him, his,
                                         start=(k == 0), stop=False)
                        nc.tensor.matmul(pa[:, 0:w], him, los,
                                         start=False, stop=False)
                        nc.tensor.matmul(pa[:, 0:w], lom, his,
                                         start=False, stop=(k == R - 1))
                    ta = sp.tile([128, 512], F32, tag="ta")
                    nc.scalar.activation(ta[:, 0:w], pa[:, 0:w], ACTF.Identity,
                                         bias=sqa_c[:, m:m + 1], scale=-2.0)
                    nc.gpsimd.tensor_tensor(ta[:, 0:w], ta[:, 0:w],
                                            sqa_rb[:, cs:ce], op=ALU.add)
                    nc.gpsimd.tensor_scalar_max(ta[:, 0:w], ta[:, 0:w], 0.0)
                    nc.scalar.activation(ta[:, 0:w], ta[:, 0:w], ACTF.Sqrt,
                                         scale=WA * WA)
                    nc.vector.scalar_tensor_tensor(
                        dist[:, m, cs:ce], ta[:, 0:w], -1.0, dist[:, m, cs:ce],
                        op0=ALU.mult, op1=ALU.subtract)
                cs0 = 128 * m
                # exact-zero the diagonal block (also clears sqrt NaNs)
                nc.gpsimd.affine_select(
                    out=dist[:, m, cs0:cs0 + 128],
                    in_=dist[:, m, cs0:cs0 + 128],
                    compare_op=ALU.not_equal, fill=0.0,
                    base=0, pattern=[[-1, 128]], channel_multiplier=1)
            # mirror lower-triangle blocks from the computed upper half
            for r in range(R):
                for c0 in range(0, r, 4):
                    nb = min(4, r - c0)
                    pt = pg.tile([128, 512], F32, tag="pa", bufs=3)
                    for j in range(nb):
                        nc.tensor.matmul(
                            pt[:, j * 128:(j + 1) * 128],
                            dist[:, c0 + j, r * 128:(r + 1) * 128],
                            ident[:], is_transpose=True)
                    nc.vector.tensor_copy(
                        dist[:, r, c0 * 128:(c0 + nb) * 128],
                        pt[:, 0:nb * 128])

            if DEBUG:
                nc.sync.dma_start(dbg["dbg_dist0"], dist[:, 0, :])

            # ---- 5-NN density via top-8 (dist is negated: top8 of -d
            # in descending order = 5 smallest d first) ----
            for m in range(R):
                dm = dist[:, m, :]
                mx8 = sp.tile([128, 8], F32, tag="mx8")
                nc.vector.max(mx8[:], dm)
                acc5 = sp.tile([128, 1], F32, tag="acc5")
                j8 = sp.tile([128, 5], F32, tag="j8")
                nc.scalar.activation(j8[:], mx8[:, 0:5], ACTF.Square,
                                     accum_out=acc5[:])
                nc.scalar.activation(den_c[:, m:m + 1], acc5[:], ACTF.Exp,
                                     scale=-1.0 / KNN)
                nc.vector.tensor_tensor(den_c[:, m:m + 1], den_c[:, m:m + 1],
                                        t_noise[:, m:m + 1], op=ALU.add)
                # per-chunk min of -d  (-> global is -dist_max)
                nc.vector.tensor_reduce(dmx_c[:, m:m + 1], dm,
                                        axis=mybir.AxisListType.X, op=ALU.min)

            # global dist max: per-partition max -> transpose -> reduce -> bcast
            dmx1 = sp.tile([128, 1], F32, tag="dmx1")
            nc.vector.tensor_reduce(dmx1[:], dmx_c[:], axis=mybir.AxisListType.X,
                                    op=ALU.min)
            ptm = p2r.tile([32, 128], F32, tag="c2r")
            nc.tensor.matmul(ptm[0:1, :], dmx1[:], ident[:], is_transpose=True)
            dmxr = sp.tile([32, 128], F32, tag="dmxr")
            nc.vector.tensor_copy(dmxr[0:1, :], ptm[0:1, :])
            dmx_s = sp.tile([32, 1], F32, tag="dmx_s")
            nc.vector.tensor_reduce(dmx_s[0:1, :], dmxr[0:1, :],
                                    axis=mybir.AxisListType.X, op=ALU.min)
            pbm = p2r.tile([128, 512], F32, tag="bc")
            nc.tensor.matmul(pbm[:, 0:1], ones1[0:1, :], dmx_s[0:1, :],
                             start=True, stop=True)
            nc.vector.tensor_copy(dmax_a[:], pbm[:, 0:1])

            # ---- dmin + score ----
            den_rb = col2rowb(den_c[:], p2r, "rowA")
            for m in range(R):
                dm = dist[:, m, :]
                msk = big([128, N])
                nc.gpsimd.tensor_scalar(msk[:], den_rb[:], den_c[:, m:m + 1],
                                        None, op0=ALU.is_gt)
                sh = big([128, N])
                nc.vector.scalar_tensor_tensor(
                    sh[:], dm, dmax_a[:], msk[:], op0=ALU.subtract, op1=ALU.mult)
                mn = sp.tile([128, 1], F32, tag="mn")
                nc.vector.tensor_reduce(mn[:], sh[:], axis=mybir.AxisListType.X,
                                        op=ALU.max)
                nc.vector.tensor_tensor(dmin_c[:, m:m + 1], mn[:], dmax_a[:],
                                        op=ALU.add)
                # sco_c holds NEGATED score: (-dmin)*den - asn
                nc.vector.scalar_tensor_tensor(
                    sco_c[:, m:m + 1], dmin_c[:, m:m + 1], den_c[:, m:m + 1],
                    t_asn[:, m:m + 1], op0=ALU.mult, op1=ALU.subtract)

            # ---- ranks ----
            sco_rb = col2rowb(sco_c[:], p2r, "rowB")
            for m in range(R):
                j1k = jp.tile([128, N], F32, tag="j1k")
                nc.vector.tensor_scalar(j1k[:], sco_rb[:], sco_c[:, m:m + 1],
                                        None, op0=ALU.is_lt, op1=ALU.add,
                                        accum_out=rank_c[:, m:m + 1])
            rankB_rb = col2rowb(rank_c[:], p2r, "rowA")
            cmask_rb = rows.tile([128, N], F32, tag="rowB")
            nc.vector.tensor_scalar(cmask_rb[:], rankB_rb[:], float(CL) - 0.5,
                                    -BIG, op0=ALU.is_gt, op1=ALU.mult)
            nc.vector.tensor_scalar_add(rankB_rb[:], rankB_rb[:], BIG)

            # ---- cluster assignment ----
            for m in range(R):
                dm = dist[:, m, :]
                ma = big([128, N])
                nc.gpsimd.tensor_tensor(ma[:], dm, cmask_rb[:], op=ALU.add)
                md = sp.tile([128, 1], F32, tag="md")
                nc.vector.tensor_reduce(md[:], ma[:], axis=mybir.AxisListType.X,
                                        op=ALU.max)
                eq = big([128, N])
                nc.gpsimd.tensor_scalar(eq[:], ma[:], md[:], None,
                                        op0=ALU.is_equal)
                cand = big([128, N])
                nc.vector.scalar_tensor_tensor(
                    cand[:], eq[:], -BIG, rankB_rb[:], op0=ALU.mult, op1=ALU.add)
                nc.vector.tensor_reduce(cid_c[:, m:m + 1], cand[:],
                                        axis=mybir.AxisListType.X, op=ALU.min)

            if DEBUG:
                cols = sp.tile([128, R * 8], F32, tag="dbgcols")
                for i, t in enumerate([sqx_c, sqa_c, den_c, dmin_c, sco_c,
                                       rank_c, cid_c, tw_c]):
                    nc.vector.tensor_copy(cols[:, i * R:(i + 1) * R], t[:])
                nc.sync.dma_start(dbg["dbg_cols"], cols[:])

        # =========== phase 9: merge ===========
        with tc.tile_pool(name="pm", bufs=1, space="PSUM") as pm:
            pm0 = pm.tile([128, COUT + 1], F32, tag="pm0")
            pm1 = pm.tile([128, COUT + 1], F32, tag="pm1")
            pms = [pm0, pm1]
            for m in range(R):
                oh = sp.tile([128, CL], F32, tag="oh")
                nc.vector.tensor_scalar(oh[:], iota256[:, 0:CL],
                                        cid_c[:, m:m + 1], None,
                                        op0=ALU.is_equal)
                rhs = sp.tile([128, COUT + 1], F32, tag="rhs")
                nc.vector.tensor_scalar_mul(rhs[:, 0:COUT], xn[:, m, :],
                                            tw_c[:, m:m + 1])
                nc.vector.tensor_copy(rhs[:, COUT:COUT + 1], tw_c[:, m:m + 1])
                for cm in range(2):
                    nc.tensor.matmul(
                        pms[cm][:], _mm(oh[:, cm * 128:(cm + 1) * 128]),
                        _mm(rhs[:]), start=(m == 0), stop=(m == R - 1))
            for cm in range(2):
                aw = sp.tile([128, 1], F32, tag="aw")
                nc.vector.tensor_scalar_add(aw[:], pms[cm][:, COUT:COUT + 1], 1e-6)
                raw = sp.tile([128, 1], F32, tag="raw")
                nc.vector.reciprocal(raw[:], aw[:])
                osb = sp.tile([128, COUT], F32, tag="osb")
                nc.vector.tensor_scalar_mul(osb[:], pms[cm][:, 0:COUT], raw[:])
                nc.sync.dma_start(d_out[cm * 128:(cm + 1) * 128, :], osb[:])

    nc.compile()
    return nc


# ------------------- host side -------------------

_NC_CACHE = {}


def _get_nc():
    key = (MM_DT, DEBUG)
    if key not in _NC_CACHE:
        _NC_CACHE[key] = build_nc()
    return _NC_CACHE[key]


def _col(v):
    """[1024] -> [128, 8] column-chunk layout (tile[p, c] = v[c*128+p])."""
    return np.ascontiguousarray(v.reshape(R, 128).T)


def make_in_maps(x, attn_in, as_out, conv_w, conv_b, skip_w, ln_g, ln_b,
                 score_w, score_b):
    import jax
    cpu = jax.local_devices(backend="cpu")[0]
    with jax.default_device(cpu):
        noise = np.asarray(jax.random.uniform(
            jax.random.key(42), (B, N), np.float32)) * 1e-6

    convw = np.ascontiguousarray(
        conv_w.transpose(2, 3, 1, 0).reshape(9, CIN, COUT).transpose(1, 0, 2)
        .reshape(CIN, 9 * COUT))
    convb = np.ascontiguousarray(conv_b.reshape(2, 128).T)
    skipw = np.ascontiguousarray(skip_w[:, :, 0].T)
    lng = np.broadcast_to(ln_g[None, :], (128, COUT)).copy()
    lnb = np.broadcast_to(ln_b[None, :], (128, COUT)).copy()
    scw = np.broadcast_to(score_w[0][None, :], (128, COUT)).copy()
    scb = np.full((128, 1), float(score_b[0]), np.float32)
    iota = np.broadcast_to(np.arange(COUT, dtype=np.float32)[None, :],
                           (128, COUT)).copy()

    in_maps = []
    for b in range(B):
        in_maps.append(dict(
            x=np.ascontiguousarray(x[b]),
            attn=np.ascontiguousarray(attn_in[b]),
            asn=_col(as_out[b].reshape(-1)),
            noise=_col(noise[b]),
            convw=convw, convb=convb, skipw=skipw,
            lng=lng, lnb=lnb, scw=scw, scb=scb, iota=iota,
        ))
    return in_maps


def kernel(x, attn_in, as_out, conv_w, conv_b, skip_w, ln_g, ln_b,
           score_w, score_b, idx_token, agg_weight):
    from concourse.bass_utils import run_bass_kernel_spmd
    nc = _get_nc()
    in_maps = make_in_maps(
        np.asarray(x, np.float32), np.asarray(attn_in, np.float32),
        np.asarray(as_out, np.float32), np.asarray(conv_w, np.float32),
        np.asarray(conv_b, np.float32), np.asarray(skip_w, np.float32),
        np.asarray(ln_g, np.float32), np.asarray(ln_b, np.float32),
        np.asarray(score_w, np.float32), np.asarray(score_b, np.float32))
    res = run_bass_kernel_spmd(nc, in_maps, core_ids=list(range(B)))
    out = np.stack([res.results[b]["out"] for b in range(B)], axis=0)
    return out
